# revision 1
# baseline (speedup 1.0000x reference)
"""Causal self-attention (B=2, N=2048, D=768, H=12) on 8 Trainium2 NeuronCores.

Sharding: data-parallel over batch (2) x tensor-parallel over head groups (4),
3 heads per core. Each core computes, for its (batch, head-group):
  GEMM1: kT/qT (transposed) and v (natural) projections from xT,
  scores^T = k @ q^T per head, exp on ScalarE (fp16 out),
  AV with a ones-augmented V giving unnormalized sa + row sums,
  normalize, PE-transpose sa -> saT, GEMM2 row-parallel -> yT partial.
All matmul operands are fp16 (fp32 PSUM accumulate); measured end-to-end
scaled relative error ~4e-4. Host shards inputs, sums the 4 per-batch
partials (the "all-reduce"), and adds the output bias fold
(bproj + bkqv_v @ Wproj — exact because softmax rows sum to 1).

Self-contained: hardcodes all shapes; no sibling imports.
"""

import os

import numpy as np

B, N, D = 2, 2048, 768
H, HD = 12, 64
HPC = 3           # heads per core
NG = 4            # head groups
NCORES = 8
P = 128
NJ = N // P       # 16 j-chunks (keys) per head
NI = N // P       # 16 i-chunks (queries)

_compiled = None  # cached compiled Bass module
last_exec_time_ns = None
last_results = None


def _build():
    import concourse.tile as tile
    import concourse.mybir as mybir
    from concourse import bacc

    f32 = mybir.dt.float32
    f16 = mybir.dt.float16
    ADD = mybir.AluOpType.add
    MULT = mybir.AluOpType.mult
    EXP = mybir.ActivationFunctionType.Exp

    nc = bacc.Bacc(
        "TRN2", target_bir_lowering=False, debug=False, num_devices=NCORES
    )

    xT_d = nc.dram_tensor("xT", [D, N], f16, kind="ExternalInput").ap()
    wkq_d = nc.dram_tensor("wkq", [D, 384], f16, kind="ExternalInput").ap()
    wv_d = nc.dram_tensor("wv", [D, 192], f16, kind="ExternalInput").ap()
    wp_d = nc.dram_tensor("wp", [64, HPC, D], f16, kind="ExternalInput").ap()
    bkq_d = nc.dram_tensor("bkq", [P, 4], f32, kind="ExternalInput").ap()
    ident_d = nc.dram_tensor("ident", [P, P], f16, kind="ExternalInput").ap()
    btri_d = nc.dram_tensor("btri", [P, P], f16, kind="ExternalInput").ap()
    yT_d = nc.dram_tensor("yT", [D, N], f32, kind="ExternalOutput").ap()

    xT_v = xT_d.rearrange("(po pi) f -> pi po f", pi=P)    # [128, 6, 2048]
    wkq_v = wkq_d.rearrange("(po pi) f -> pi po f", pi=P)  # [128, 6, 384]
    wv_v = wv_d.rearrange("(po pi) f -> pi po f", pi=P)    # [128, 6, 192]
    yT_v = yT_d.rearrange("(po pi) f -> pi po f", pi=P)    # [128, 6, 2048]

    with tile.TileContext(nc) as tc:
        import contextlib

        ctx = contextlib.ExitStack()
        with ctx:
            const = ctx.enter_context(tc.tile_pool(name="const", bufs=1))
            big = ctx.enter_context(tc.tile_pool(name="bigbufs", bufs=1))
            work = ctx.enter_context(tc.tile_pool(name="work", bufs=3))
            ypool = ctx.enter_context(tc.tile_pool(name="ypool", bufs=3))
            psum_sc = ctx.enter_context(
                tc.tile_pool(name="psum_sc", bufs=2, space="PSUM")
            )
            psum_gemm = ctx.enter_context(
                tc.tile_pool(name="psum_gemm", bufs=2, space="PSUM")
            )
            psum_av = ctx.enter_context(
                tc.tile_pool(name="psum_av", bufs=2, space="PSUM")
            )

            # ---- constants / weights to SBUF ----
            ident_t = const.tile([P, P], f16, name="ident_t")
            nc.sync.dma_start(ident_t[:], ident_d)
            btri_t = const.tile([P, P], f16, name="btri_t")
            nc.sync.dma_start(btri_t[:], btri_d)
            bkq_t = const.tile([P, 4], f32, name="bkq_t")
            nc.sync.dma_start(bkq_t[:], bkq_d)
            # wkq split per dc chunk so the first GEMM1 accumulation starts
            # as soon as chunk 0 lands
            wkq_ts = [
                const.tile([P, 384], f16, name=f"wkq_t{dc}") for dc in range(6)
            ]
            for dc in range(6):
                nc.sync.dma_start(wkq_ts[dc][:], wkq_v[:, dc, :])
            wv_t = const.tile([P, 6, 192], f16, name="wv_t")
            nc.sync.dma_start(wv_t[:], wv_v)
            wp_t = const.tile([64, HPC, D], f16, name="wp_t")
            nc.sync.dma_start(wp_t[:], wp_d)
            # PE warmup: ~4us of dummy matmuls on a zeroed scratch while the
            # input DMAs land, so the HAM clock-gate opens before GEMM1
            wscr = const.tile([P, 512], f16, name="wscr")
            nc.vector.memset(wscr[:], 0.0)
            wps = psum_gemm.tile([P, 512], f32, tag="ps512", name="wps")
            for _ in range(10):
                nc.tensor.matmul(
                    wps[:, 0:512], wscr[:, 0:128], wscr[:], start=True, stop=True
                )

            # xT as 6 per-chunk tiles with separate DMAs: spreads across DMA
            # queues AND lets GEMM1's first accumulation start on chunk 0
            xT_ts = [big.tile([P, N], f16, name=f"xT_t{dc}") for dc in range(6)]
            for dc in range(6):
                nc.sync.dma_start(xT_ts[dc][:], xT_v[:, dc, :])

            # kT/qT split per chunk so head-0/1 strips can start before the
            # 64-wide h2 projections finish
            kT0 = big.tile([P, N], f16, name="kT0")
            qT0 = big.tile([P, N], f16, name="qT0")
            kT1 = big.tile([64, N], f16, name="kT1")
            qT1 = big.tile([64, N], f16, name="qT1")
            kqT = [(kT0, qT0), (kT1, qT1)]
            vaug = big.tile([P, NJ, HPC, 65], f16, name="vaug")
            # saT split per 512-query slice so GEMM2 starts as slices complete
            saTs = [
                big.tile([64, HPC, 512], f16, name=f"saT{i}") for i in range(4)
            ]

            # ---- GEMM1-kq: 3 chunks of 128 output channels ----
            # wkq cols: [k01 (128) | q01 (128) | k2|q2 (128)]; q2 lands on
            # psum partitions 64:128 and moves to qT1 base-0 via SBUF DMA
            q2st = big.tile([P, N], f16, name="q2st")
            chunks = [(kT0, 0), (qT0, 128), (None, 256)]

            def emit_gemm1_kq(cis):
                for ci in cis:
                    dst, lo = chunks[ci]
                    for isl in range(4):
                        ps = psum_gemm.tile([P, 512], f32, tag="ps512", name="ps_kq")
                        for dc in range(6):
                            nc.tensor.matmul(
                                ps[:, 0:512],
                                wkq_ts[dc][:, lo : lo + 128],
                                xT_ts[dc][:, 512 * isl : 512 * isl + 512],
                                start=(dc == 0),
                                stop=(dc == 5),
                            )
                        sl = slice(512 * isl, 512 * isl + 512)
                        if ci < 2:
                            nc.vector.tensor_scalar(
                                dst[:, sl], ps[:, 0:512],
                                bkq_t[:, ci : ci + 1], None, op0=ADD,
                            )
                        else:
                            nc.vector.tensor_scalar(
                                kT1[0:64, sl], ps[0:64, 0:512],
                                bkq_t[0:64, 2:3], None, op0=ADD,
                            )
                            nc.vector.tensor_scalar(
                                q2st[64:128, sl], ps[64:128, 0:512],
                                bkq_t[64:128, 3:4], None, op0=ADD,
                            )
                            nc.sync.dma_start(qT1[0:64, sl], q2st[64:128, sl])

            def emit_gemm1_v():
                nc.vector.memset(vaug[:, :, :, 64:65], 1.0)
                for ic in range(NI):
                    ps = psum_gemm.tile([P, 512], f32, tag="ps512", name="ps_v")
                    for dc in range(6):
                        nc.tensor.matmul(
                            ps[:, 0:192],
                            xT_ts[dc][:, 128 * ic : 128 * ic + 128],
                            wv_t[:, dc, :],
                            start=(dc == 0),
                            stop=(dc == 5),
                        )
                    nc.vector.tensor_copy(
                        out=vaug[:, ic, :, 0:64],
                        in_=ps[:, 0:192].rearrange("p (h d) -> p h d", h=HPC),
                    )

            # ---- attention ----
            # scores/exp strips feed a stationary-V AV:
            #   saT_aug[65, i] = vaug^T @ expT  (row 64 = softmax denominators)
            # accumulated over j-chunks with 512-wide moving operands. Each
            # head's 16 strips are emitted as one dense PE burst; ACT drains
            # the exps behind it and the AV stages chase per strip.
            all_strips = [[None] * NJ for _ in range(HPC)]

            def make_emit_strip(h):
                if h < 2:
                    cc, pb = 0, 64 * h
                else:
                    cc, pb = 1, 0
                kTc, qTc = kqT[cc]
                strips = all_strips[h]

                def emit_strip(jc, kTc=kTc, qTc=qTc, pb=pb, strips=strips):
                    i0 = 128 * jc
                    W = N - i0
                    strip = work.tile(
                        [P, W], f16, tag=f"expT{jc}", bufs=3, name=f"expT{jc}"
                    )
                    for s0 in range(0, W, 1024):
                        sw = min(1024, W - s0)
                        ps = psum_sc.tile([P, 1024], f32, tag="sc", name="ps_s")
                        diag = s0 == 0  # first 128 cols are the diagonal block
                        for sub in range(0, sw, 512):
                            ssw = min(512, sw - sub)
                            chained = diag and sub == 0
                            nc.tensor.matmul(
                                ps[:, sub : sub + ssw],
                                kTc[pb : pb + 64, i0 : i0 + 128],
                                qTc[pb : pb + 64,
                                    i0 + s0 + sub : i0 + s0 + sub + ssw],
                                start=True,
                                stop=(not chained),
                            )
                            if chained:
                                # causal mask: accumulate -30000 above the
                                # diagonal ((ident^T @ btri)[j, i] = btri[j, i])
                                nc.tensor.matmul(
                                    ps[:, 0:128],
                                    ident_t[:],
                                    btri_t[:],
                                    start=False,
                                    stop=True,
                                )
                        nc.scalar.activation(
                            strip[:, s0 : s0 + sw], ps[:, 0:sw], EXP, scale=0.125
                        )
                    strips[jc] = strip

                return emit_strip

            def make_emit_av(h):
                strips = all_strips[h]

                def emit_av(iseg, h=h, strips=strips):
                    # saT_aug for queries i in [512*iseg, 512*iseg+512)
                    ps2 = psum_av.tile([65, 512], f32, tag="av", name="ps2")
                    jmax = 4 * iseg + 3
                    for jc in range(jmax + 1):
                        off = 512 * iseg - 128 * jc  # strip-local col of i0
                        lo = max(0, off)
                        w = 512 - (lo - off)
                        nc.tensor.matmul(
                            ps2[0:65, 512 - w : 512],
                            vaug[:, jc, h, :],
                            strips[jc][:, lo : lo + w],
                            start=(jc == 0),
                            stop=(jc == jmax),
                        )
                    # normalization: recip of the sums row, broadcast over
                    # the 64 sa partitions, multiply.
                    srow = work.tile([1, 512], f32, tag="srow", bufs=2, name="srow")
                    nc.vector.tensor_copy(out=srow[:], in_=ps2[64:65, :])
                    rrow = work.tile([1, 512], f32, tag="rrow", bufs=2, name="rrow")
                    nc.vector.reciprocal_approx_fast(out=rrow[:], in_=srow[:])
                    rbc = work.tile([64, 512], f32, tag="rbc", bufs=2, name="rbc")
                    nc.gpsimd.partition_broadcast(rbc[:], rrow[:])
                    nc.vector.tensor_tensor(
                        saTs[iseg][:, h, :],
                        ps2[0:64, :],
                        rbc[:],
                        MULT,
                    )

                return emit_av

            emit_strips = [make_emit_strip(h) for h in range(HPC)]
            emit_avs = [make_emit_av(h) for h in range(HPC)]

            def emit_gemm2(isl):
                for oc in range(6):
                    ps = psum_gemm.tile([P, 512], f32, tag="ps512", name="ps_y")
                    for h in range(HPC):
                        nc.tensor.matmul(
                            ps[:, 0:512],
                            wp_t[:, h, 128 * oc : 128 * oc + 128],
                            saTs[isl][:, h, :],
                            start=(h == 0),
                            stop=(h == HPC - 1),
                        )
                    yst = ypool.tile([P, 512], f32, tag="yst", name="yst")
                    nc.vector.tensor_copy(out=yst[:], in_=ps[:, 0:512])
                    nc.sync.dma_start(
                        yT_v[:, oc, 512 * isl : 512 * isl + 512], yst[:]
                    )

            # phase order: h0/h1 projections, then head-0 strips start ACT as
            # early as possible; the remaining PE phases stream behind while
            # ACT drains exps; AVs chase per head; GEMM2+output per i-slice
            # interleaves with the last head's AV.
            emit_gemm1_kq([0, 1])
            for jc in range(NJ):
                emit_strips[0](jc)
            emit_gemm1_kq([2])
            emit_gemm1_v()
            for jc in range(NJ):
                emit_strips[1](jc)
            for jc in range(NJ):
                emit_strips[2](jc)
            for iseg in range(4):
                emit_avs[0](iseg)
            for iseg in range(4):
                emit_avs[1](iseg)
            for iseg in range(4):
                emit_avs[2](iseg)
                emit_gemm2(iseg)

    nc.compile()
    return nc


def _host_prep(x, Wkqv, bkqv, Wproj, bproj):
    f16 = np.float16
    Wk = Wkqv[:, 0:D]
    Wq = Wkqv[:, D : 2 * D]
    Wv = Wkqv[:, 2 * D : 3 * D]
    bk = bkqv[0:D]
    bq = bkqv[D : 2 * D]
    bv = bkqv[2 * D : 3 * D]
    out_bias = (bproj + bv @ Wproj).astype(np.float32)  # softmax rows sum to 1

    ident = np.eye(P, dtype=f16)
    # btri[k, i] = -30000 where k > i: accumulated into scoresT diag blocks,
    # exp((s - 30000) * 0.125) underflows to exactly 0 in fp16.
    btri = (np.tril(np.full((P, P), -30000.0, np.float32), -1)).astype(f16)

    in_maps = []
    for b in range(B):
        xT = np.ascontiguousarray(x[b].T.astype(f16))
        for g in range(NG):
            hs = [HPC * g + i for i in range(HPC)]
            wk = [np.asarray(Wk[:, HD * h : HD * h + HD]) for h in hs]
            wq = [np.asarray(Wq[:, HD * h : HD * h + HD]) for h in hs]
            wv = [np.asarray(Wv[:, HD * h : HD * h + HD]) for h in hs]
            wkq = np.concatenate(
                [wk[0], wk[1], wq[0], wq[1], wk[2], wq[2]], axis=1
            ).astype(f16)
            wv_c = np.concatenate(wv, axis=1).astype(f16)
            wp = np.stack(
                [Wproj[HD * h : HD * h + HD, :] for h in hs], axis=1
            ).astype(f16)  # [64, 3, 768]
            bkq = np.zeros((P, 4), np.float32)
            bkq[:, 0] = np.concatenate(
                [bk[HD * hs[0] : HD * hs[0] + HD], bk[HD * hs[1] : HD * hs[1] + HD]]
            )
            bkq[:, 1] = np.concatenate(
                [bq[HD * hs[0] : HD * hs[0] + HD], bq[HD * hs[1] : HD * hs[1] + HD]]
            )
            bkq[0:64, 2] = bk[HD * hs[2] : HD * hs[2] + HD]
            bkq[64:128, 3] = bq[HD * hs[2] : HD * hs[2] + HD]
            in_maps.append(
                dict(xT=xT, wkq=wkq, wv=wv_c, wp=wp, bkq=bkq,
                     ident=ident, btri=btri)
            )
    return in_maps, out_bias


def kernel(x, Wkqv, bkqv, Wproj, bproj):
    global _compiled, last_exec_time_ns, last_results
    import concourse.bass_utils as bass_utils

    x = np.asarray(x, np.float32)
    Wkqv = np.asarray(Wkqv, np.float32)
    bkqv = np.asarray(bkqv, np.float32)
    Wproj = np.asarray(Wproj, np.float32)
    bproj = np.asarray(bproj, np.float32)

    if _compiled is None:
        _compiled = _build()
    nc = _compiled

    in_maps, out_bias = _host_prep(x, Wkqv, bkqv, Wproj, bproj)

    trace = os.environ.get("BASS_KERNEL_TRACE", "0") == "1"
    res = bass_utils.run_bass_kernel_spmd(
        nc, in_maps, core_ids=list(range(NCORES)), trace=trace
    )
    last_exec_time_ns = res.exec_time_ns
    last_results = res

    out = np.zeros((B, N, D), np.float32)
    for b in range(B):
        acc = np.zeros((D, N), np.float32)
        for g in range(NG):
            acc += res.results[b * NG + g]["yT"]
        out[b] = acc.T + out_bias
    return out

